# revision 19
# baseline (speedup 1.0000x reference)
"""Causal self-attention (B=1, T=4096, C=768, H=12, D=64) on 8 TRN2 NeuronCores.

Sharding: 8 cores = 4 head-groups (3 heads each) x 2 sequence-groups.
Core c: heads [3*hg, 3*hg+2] where hg=c//2; handles q-chunks of 256 rows,
global chunk g = 2*j + s (s=c%2, j=0..7) -- interleaving balances the causal
triangle so every core runs an identical instruction stream (SPMD), with the
boundary masks supplied as per-core data.

Every matmul runs in fp16 single-pass (fp32 PSUM accumulation; all operands
O(1) so fp16 range is ample).  Flash-style attention in S^T = K^T q
orientation with K=64 contraction (matmul cost depends only on the moving
size, so no 128-partition packing is needed for lhsT/rhs).  V is produced
directly in [k, d] layout (lhsT = x^T chunk, rhs = W_v), avoiding PE
transposes; a per-head ones column interleaved in the V tile makes the
softmax denominator fall out of the same PV matmuls.  K-bias is dropped
(softmax shift invariance, exact); V-bias is folded into b_proj on the host
(linearity, exact).  Causality = block skipping + 0/1 boundary masks.
Host sums the 4 head-group partial projections.
"""
import numpy as np

T, C, H, D = 4096, 768, 12, 64
NH = 3          # heads per core
QC = 256        # q rows per slot
P = 128
NKB = T // P    # 32 k-blocks
NCH = 8         # phase-1 column chunks
CHW = T // NCH  # 512 cols per chunk
NSLOT = T // (2 * QC)   # 8 q-slots per core
TQ = NSLOT * QC         # 2048 q rows per core

_nc_cache = {}


def split_multi_waits(nc):
    """Walrus here accepts only one sync wait per instruction: hoist extras
    onto standalone InstEventSemaphore instructions on the same engine."""
    import concourse.mybir as mybir
    n_split = 0
    for f in nc.m.functions:
        for bb in f.blocks:
            new_insts = []
            for inst in bb.instructions:
                si = inst.sync_info
                if si is not None and len(si.on_wait) > 1:
                    for w in si.on_wait[:-1]:
                        nop = mybir.InstEventSemaphore(
                            name=nc.get_next_instruction_name(), ins=[], outs=[])
                        nop.engine = inst.engine
                        nop.sync_info = mybir.SyncInfo(on_wait=[w], on_update=[])
                        nc.register_instruction(nop)
                        new_insts.append(nop)
                        n_split += 1
                    si.on_wait = si.on_wait[-1:]
                new_insts.append(inst)
            bb.instructions[:] = new_insts
    return n_split


def build_nc():
    import concourse.bass as bass
    import concourse.mybir as mybir
    import concourse.tile as tile
    from contextlib import ExitStack

    f32r = mybir.dt.float32r
    f32 = mybir.dt.float32
    f16 = mybir.dt.float16
    EXP = mybir.ActivationFunctionType.Exp
    CPY = mybir.ActivationFunctionType.Copy
    ADD = mybir.AluOpType.add

    nc = bass.Bass(trn_type="TRN2")
    xt16 = nc.dram_tensor("xt16", [C, T], f16, kind="ExternalInput")
    xtq16 = nc.dram_tensor("xtq16", [C, TQ], f16, kind="ExternalInput")
    wk01 = nc.dram_tensor("wk01", [C, 2 * D], f16, kind="ExternalInput")
    wk2 = nc.dram_tensor("wk2", [C, D], f16, kind="ExternalInput")
    wv3 = nc.dram_tensor("wv3", [C, NH * D], f16, kind="ExternalInput")
    wq01 = nc.dram_tensor("wq01", [C, 2 * D], f16, kind="ExternalInput")
    wq2 = nc.dram_tensor("wq2", [C, D], f16, kind="ExternalInput")
    wpj16 = nc.dram_tensor("wpj16", [NH, D, C], f16, kind="ExternalInput")
    bq = nc.dram_tensor("bq", [P, 2], f32, kind="ExternalInput")
    mask = nc.dram_tensor("mask", [NSLOT, P, 1024], f16, kind="ExternalInput")
    out = nc.dram_tensor("out", [TQ, C], f32, kind="ExternalOutput")

    with tile.TileContext(nc) as tc, ExitStack() as ctx:
        singles = ctx.enter_context(tc.tile_pool(name="singles", bufs=1))
        xchp = ctx.enter_context(tc.tile_pool(name="xch", bufs=3))
        xqp = ctx.enter_context(tc.tile_pool(name="xq", bufs=2))
        qtp = ctx.enter_context(tc.tile_pool(name="qt", bufs=2))
        mtp = ctx.enter_context(tc.tile_pool(name="mt", bufs=2))
        ptp = ctx.enter_context(tc.tile_pool(name="pt", bufs=3))
        rbp = ctx.enter_context(tc.tile_pool(name="rb", bufs=2))
        ytp = ctx.enter_context(tc.tile_pool(name="yt", bufs=2))
        ostp = ctx.enter_context(tc.tile_pool(name="ost", bufs=2))
        psg = ctx.enter_context(tc.tile_pool(name="psg", bufs=2, space="PSUM"))
        psy = ctx.enter_context(tc.tile_pool(name="psy", bufs=2, space="PSUM"))
        psm = ctx.enter_context(tc.tile_pool(name="psm", bufs=2, space="PSUM"))

        ones_f = singles.tile([1, 64], f32)
        nc.vector.memset(ones_f, 1.0)
        ones64 = singles.tile([1, 64], f32r)
        nc.vector.tensor_copy(ones64, ones_f)
        bln16 = singles.tile([P, 1], f32)   # exp bias -ln(16): fp16 headroom
        nc.vector.memset(bln16, -2.772588722239781)
        # head-select matrices: e3[:, 64h:64h+64] has row 32h all-ones, rest 0;
        # bc_h = e3_h^T @ rdens broadcasts head h's denominator row (base-0
        # rhs; AP partition starts are restricted to 0/32/64, hence the 32h
        # row placement)
        e3f = singles.tile([65, NH * 64], f32)
        nc.vector.memset(e3f, 0.0)
        for h in range(NH):
            nc.vector.memset(e3f[32 * h:32 * h + 1, 64 * h:64 * h + 64], 1.0)
        e3 = singles.tile([65, NH * 64], f32r)
        nc.vector.tensor_copy(e3, e3f)

        wk01_t = singles.tile([P, 6, 2 * D], f16)
        wk2_t = singles.tile([P, 6, D], f16)
        wv3_t = singles.tile([P, 6, NH * D], f16)
        wq01_t = singles.tile([P, 6, 2 * D], f16)
        wq2_t = singles.tile([P, 6, D], f16)
        for c in range(6):
            rows = slice(P * c, P * c + P)
            nc.sync.dma_start(wk01_t[:, c], wk01[rows, :])
            nc.sync.dma_start(wk2_t[:, c], wk2[rows, :])
            nc.sync.dma_start(wv3_t[:, c], wv3[rows, :])
            nc.sync.dma_start(wq01_t[:, c], wq01[rows, :])
            nc.sync.dma_start(wq2_t[:, c], wq2[rows, :])
        wpj_t = []
        for h in range(NH):
            w1 = singles.tile([D, C], f16, tag=f"wpj{h}", name=f"wpj{h}")
            nc.sync.dma_start(w1, wpj16[h])
            wpj_t.append(w1)
        bq_t = singles.tile([P, 2], f32)
        nc.sync.dma_start(bq_t, bq[:, :])

        # K^T in [d, k] layout: heads 0,1 stacked on 128 partitions; head 2 alone
        kt01 = singles.tile([P, T], f16, tag="kt01", name="kt01")
        kt2 = singles.tile([D, T], f16, tag="kt2", name="kt2")
        # V in [k, d] layout, per k-block: [v0(64) 1 | v1(64) 1 | v2(64) 1]
        vp3 = singles.tile([P, NKB, NH * 65], f16, tag="vp3", name="vp3")
        nc.vector.memset(vp3, 1.0)  # ones cols survive; V cols overwritten

        # ---- Phase 1: K^T and V from x^T, in 512-col chunks ----
        for ch in range(NCH):
            cols = slice(CHW * ch, CHW * (ch + 1))
            xch = xchp.tile([P, 6, CHW], f16, tag="xch", name="xch")
            for c in range(6):
                nc.sync.dma_start(xch[:, c], xt16[P * c:P * c + P, cols])
            ps01 = psg.tile([P, CHW], f32, tag="sg", name="ps01")
            for c in range(6):
                nc.tensor.matmul(ps01, wk01_t[:, c], xch[:, c],
                                 start=(c == 0), stop=(c == 5))
            nc.scalar.activation(kt01[:, cols], ps01, CPY)
            ps2 = psg.tile([D, CHW], f32, tag="sg", name="ps2")
            for c in range(6):
                nc.tensor.matmul(ps2, wk2_t[:, c], xch[:, c],
                                 start=(c == 0), stop=(c == 5))
            nc.vector.tensor_copy(kt2[:, cols], ps2)
            for i in range(CHW // P):
                kb = (CHW // P) * ch + i
                psv = psy.tile([P, NH * D], f32, tag="yacc", name="psv")
                for c in range(6):
                    nc.tensor.matmul(psv, xch[:, c, P * i:P * (i + 1)],
                                     wv3_t[:, c], start=(c == 0), stop=(c == 5))
                nc.vector.tensor_copy(
                    vp3[:, kb].rearrange("p (h c) -> p h c", c=65)[:, :, 0:64],
                    psv.rearrange("p (h c) -> p h c", c=64))

        # ---- Phase 2: per q-slot: Q^T, attention, projection ----
        for j in range(NSLOT):
            xq = xqp.tile([P, 6, QC], f16, tag="xq", name="xq")
            for c in range(6):
                nc.sync.dma_start(xq[:, c], xtq16[P * c:P * c + P, QC * j:QC * (j + 1)])
            psq01 = psm.tile([P, QC], f32, tag="psm", name="psq01")
            for c in range(6):
                nc.tensor.matmul(psq01, wq01_t[:, c], xq[:, c],
                                 start=(c == 0), stop=(c == 5))
            qt01 = qtp.tile([P, QC], f16, tag="qt01", name="qt01")
            nc.vector.tensor_scalar(qt01, psq01, bq_t[:, 0:1], None, ADD)
            psq2 = psm.tile([D, QC], f32, tag="psm", name="psq2")
            for c in range(6):
                nc.tensor.matmul(psq2, wq2_t[:, c], xq[:, c],
                                 start=(c == 0), stop=(c == 5))
            qt2 = qtp.tile([D, QC], f16, tag="qt2", name="qt2")
            nc.vector.tensor_scalar(qt2, psq2, bq_t[0:D, 1:2], None, ADD)

            mt = mtp.tile([P, 1024], f16, tag="mt", name="mt")
            nc.sync.dma_start(mt, mask[j])

            qts = [qt01[0:D], qt01[D:P], qt2]
            kts = [kt01[0:D], kt01[D:P], kt2]
            # unnormalized y' and denominators, staged to SBUF so the yacc
            # PSUM frees early; exp is biased by -ln16 to keep y' in fp16 range
            # (dens rows at partitions 0/32/64; filler lanes memset to 1.0 so
            # the reciprocal stays finite for the e3 selector matmul)
            dens = rbp.tile([65, QC], f32, tag="dens", name="dens")
            nc.vector.memset(dens, 1.0)
            ytu_t = []
            for h in range(NH):
                yacc = psy.tile([65, QC], f32, tag="yacc", name="yacc")

                def s_group(g):
                    sg = psg.tile([P, 1024], f32, tag="sg", name="sg")
                    for i in range(4):
                        kb = 4 * g + i
                        nc.tensor.matmul(sg[:, QC * i:QC * (i + 1)],
                                         kts[h][:, P * kb:P * (kb + 1)],
                                         qts[h], start=True, stop=True)
                    return sg

                # software pipeline: issue S(g+1) before PV(g) so the PE has
                # work while ACT runs exp(g)
                sg_cur = s_group(0)
                for g in range(j + 1):
                    sg_next = s_group(g + 1) if g < j else None
                    pt = ptp.tile([P, 1024], f16, tag="pt", name="pt")
                    nc.scalar.activation(pt, sg_cur, EXP, scale=0.125,
                                         bias=bln16[:, 0:1])
                    if g == j:
                        nc.vector.tensor_mul(pt, pt, mt)
                    for i in range(4):
                        kb = 4 * g + i
                        nc.tensor.matmul(yacc, vp3[:, kb, 65 * h:65 * h + 65],
                                         pt[:, QC * i:QC * (i + 1)],
                                         start=(g == 0 and i == 0),
                                         stop=(g == j and i == 3))
                    sg_cur = sg_next
                nc.vector.tensor_copy(dens[32 * h:32 * h + 1], yacc[64:65])
                ytu = ytp.tile([64, QC], f16, tag=f"ytu{h}", name=f"ytu{h}")
                nc.vector.tensor_copy(ytu, yacc[0:64])
                ytu_t.append(ytu)
            # normalize all 3 heads with one batched reciprocal; replicate
            # each denominator row over 64 partitions via a K=1 ones matmul
            rdens32 = rbp.tile([65, QC], f32, tag="rdens32", name="rdens32")
            nc.vector.reciprocal(rdens32, dens)
            rdens = rbp.tile([65, QC], f32r, tag="rdens", name="rdens")
            nc.vector.tensor_copy(rdens, rdens32)
            yt_t = []
            for h in range(NH):
                bc = psm.tile([64, QC], f32, tag="psm", name="bc")
                nc.tensor.matmul(bc, e3[:, 64 * h:64 * h + 64], rdens,
                                 start=True, stop=True)
                yt = ytp.tile([64, QC], f16, tag=f"yt{h}", name=f"yt{h}")
                nc.vector.tensor_mul(yt, ytu_t[h], bc)
                yt_t.append(yt)

            ost = ostp.tile([P, 2, C], f32, tag="ost", name="ost")
            for qb in range(2):
                for (n0, nw) in [(0, 512), (512, 256)]:
                    pp = psm.tile([P, nw], f32, tag="psm", name="pp")
                    for h in range(NH):
                        nc.tensor.matmul(pp, yt_t[h][:, P * qb:P * (qb + 1)],
                                         wpj_t[h][:, n0:n0 + nw],
                                         start=(h == 0), stop=(h == NH - 1))
                    nc.vector.tensor_copy(ost[:, qb, n0:n0 + nw], pp)
            for qb in range(2):
                nc.sync.dma_start(
                    out[QC * j + P * qb:QC * j + P * (qb + 1), :], ost[:, qb])

    split_multi_waits(nc)
    return nc


def make_in_maps(x, W_qkv, b_qkv, W_proj):
    """Shard the full inputs into the 8 per-core input maps."""
    xT = np.ascontiguousarray(x.reshape(T, C).T).astype(np.float32)
    xT16 = xT.astype(np.float16)

    kk = np.arange(P)
    qq = np.arange(QC)
    in_maps = []
    for core in range(8):
        hg, s = core // 2, core % 2
        heads = [3 * hg + i for i in range(NH)]
        wk = [W_qkv[:, C + 64 * h:C + 64 * h + 64] for h in heads]
        wv = [W_qkv[:, 2 * C + 64 * h:2 * C + 64 * h + 64] for h in heads]
        wq = [W_qkv[:, 64 * h:64 * h + 64] for h in heads]
        wk01_c = np.concatenate(wk[0:2], axis=1).astype(np.float16)
        wk2_c = np.ascontiguousarray(wk[2]).astype(np.float16)
        wv3_c = np.concatenate(wv, axis=1).astype(np.float16)
        wq01_c = np.concatenate(wq[0:2], axis=1).astype(np.float16)
        wq2_c = np.ascontiguousarray(wq[2]).astype(np.float16)
        wpj_c = np.stack([W_proj[64 * h:64 * h + 64, :] for h in heads]
                         ).astype(np.float16)

        bq_c = np.zeros((P, 2), np.float32)
        bq_c[0:64, 0] = b_qkv[64 * heads[0]:64 * heads[0] + 64]
        bq_c[64:P, 0] = b_qkv[64 * heads[1]:64 * heads[1] + 64]
        bq_c[0:64, 1] = b_qkv[64 * heads[2]:64 * heads[2] + 64]

        qcols = np.concatenate(
            [np.arange(QC * (2 * j + s), QC * (2 * j + s) + QC)
             for j in range(NSLOT)])
        xtq_16 = np.ascontiguousarray(xT16[:, qcols])

        mask_c = np.zeros((NSLOT, P, 1024), np.float32)
        for j in range(NSLOT):
            q0 = QC * (2 * j + s)
            for i in range(4):
                k0 = P * (4 * j + i)
                mask_c[j, :, QC * i:QC * (i + 1)] = (
                    (k0 + kk[:, None]) <= (q0 + qq[None, :]))

        in_maps.append({
            "xt16": xT16, "xtq16": xtq_16,
            "wk01": wk01_c, "wk2": wk2_c, "wv3": wv3_c,
            "wq01": wq01_c, "wq2": wq2_c, "wpj16": wpj_c,
            "bq": bq_c, "mask": mask_c.astype(np.float16),
        })
    return in_maps


def unshard(results, b_qkv, W_proj, b_proj):
    out = np.zeros((T, C), np.float64)
    for core in range(8):
        s = core % 2
        r = results[core]["out"].astype(np.float64)
        for j in range(NSLOT):
            g0 = QC * (2 * j + s)
            out[g0:g0 + QC] += r[QC * j:QC * (j + 1)]
    # V-bias folded here: (y + b_v) @ W_proj = y @ W_proj + b_v @ W_proj
    b_eff = b_proj.astype(np.float64) + (
        b_qkv[2 * C:3 * C].astype(np.float64) @ W_proj.astype(np.float64))
    out += b_eff
    return out.astype(np.float32).reshape(1, T, C)


_last_result = {}


def kernel(x, mask, W_qkv, b_qkv, W_proj, b_proj):
    from concourse.bass_utils import run_bass_kernel_spmd
    x = np.asarray(x, np.float32)
    W_qkv = np.asarray(W_qkv, np.float32)
    b_qkv = np.asarray(b_qkv, np.float32)
    W_proj = np.asarray(W_proj, np.float32)
    b_proj = np.asarray(b_proj, np.float32)

    if "nc" not in _nc_cache:
        _nc_cache["nc"] = build_nc()
    nc = _nc_cache["nc"]
    in_maps = make_in_maps(x, W_qkv, b_qkv, W_proj)
    import os
    kwargs = {}
    if os.environ.get("BASS_KERNEL_TRACE"):
        kwargs = dict(trace=True, trace_cores=list(range(8)))
    res = run_bass_kernel_spmd(nc, in_maps, core_ids=list(range(8)), **kwargs)
    _last_result["res"] = res
    return unshard([r for r in res.results], b_qkv, W_proj, b_proj)


# revision 20
# speedup vs baseline: 1.2388x; 1.2388x over previous
"""Causal self-attention (B=1, T=4096, C=768, H=12, D=64) on 8 TRN2 NeuronCores.

Sharding: 8 cores = 4 head-groups (3 heads each) x 2 sequence-groups.
Core c: heads [3*hg, 3*hg+2] where hg=c//2; handles q-chunks of 256 rows,
global chunk g = 2*j + s (s=c%2, j=0..7) -- interleaving balances the causal
triangle so every core runs an identical instruction stream (SPMD), with the
boundary masks supplied as per-core data.

Every matmul runs in fp16 single-pass (fp32 PSUM accumulation; all operands
O(1) so fp16 range is ample).  Flash-style attention in S^T = K^T q
orientation with K=64 contraction (matmul cost depends only on the moving
size, so no 128-partition packing is needed for lhsT/rhs).  V is produced
directly in [k, d] layout (lhsT = x^T chunk, rhs = W_v), avoiding PE
transposes; a per-head ones column interleaved in the V tile makes the
softmax denominator fall out of the same PV matmuls.  K-bias is dropped
(softmax shift invariance, exact); V-bias is folded into b_proj on the host
(linearity, exact).  Causality = block skipping + 0/1 boundary masks.
The attention inner loop is ACT(exp)-paced, so each slot's normalize +
projection is emitted inside the NEXT slot's attention stream to keep the
PE queue fed.  Host sums the 4 head-group partial projections.
"""
import numpy as np

T, C, H, D = 4096, 768, 12, 64
NH = 3          # heads per core
QC = 256        # q rows per slot
P = 128
NKB = T // P    # 32 k-blocks
NBC = 4         # phase-1 big DMA chunks
BCW = T // NBC  # 1024 cols per big chunk
NSLOT = T // (2 * QC)   # 8 q-slots per core
TQ = NSLOT * QC         # 2048 q rows per core
# fused weight-tensor column layout: [wk01 | wk2 | wv3 | wq01 | wq2]
WK01, WK2, WV3, WQ01, WQ2 = 0, 128, 192, 384, 512
WALL = 576

_nc_cache = {}


def split_multi_waits(nc):
    """Walrus here accepts only one sync wait per instruction: hoist extras
    onto standalone InstEventSemaphore instructions on the same engine."""
    import concourse.mybir as mybir
    n_split = 0
    for f in nc.m.functions:
        for bb in f.blocks:
            new_insts = []
            for inst in bb.instructions:
                si = inst.sync_info
                if si is not None and len(si.on_wait) > 1:
                    for w in si.on_wait[:-1]:
                        nop = mybir.InstEventSemaphore(
                            name=nc.get_next_instruction_name(), ins=[], outs=[])
                        nop.engine = inst.engine
                        nop.sync_info = mybir.SyncInfo(on_wait=[w], on_update=[])
                        nc.register_instruction(nop)
                        new_insts.append(nop)
                        n_split += 1
                    si.on_wait = si.on_wait[-1:]
                new_insts.append(inst)
            bb.instructions[:] = new_insts
    return n_split


def build_nc():
    import concourse.bass as bass
    import concourse.mybir as mybir
    import concourse.tile as tile
    from contextlib import ExitStack

    f32r = mybir.dt.float32r
    f32 = mybir.dt.float32
    f16 = mybir.dt.float16
    EXP = mybir.ActivationFunctionType.Exp
    CPY = mybir.ActivationFunctionType.Copy
    ADD = mybir.AluOpType.add

    nc = bass.Bass(trn_type="TRN2")
    xt16 = nc.dram_tensor("xt16", [C, T], f16, kind="ExternalInput")
    xtq16 = nc.dram_tensor("xtq16", [C, TQ], f16, kind="ExternalInput")
    wall = nc.dram_tensor("wall", [C, WALL], f16, kind="ExternalInput")
    wpj16 = nc.dram_tensor("wpj16", [NH, D, C], f16, kind="ExternalInput")
    bq = nc.dram_tensor("bq", [P, 2], f32, kind="ExternalInput")
    mask = nc.dram_tensor("mask", [NSLOT, P, 1024], f16, kind="ExternalInput")
    out = nc.dram_tensor("out", [TQ, C], f32, kind="ExternalOutput")

    with tile.TileContext(nc) as tc, ExitStack() as ctx:
        singles = ctx.enter_context(tc.tile_pool(name="singles", bufs=1))
        xchp = ctx.enter_context(tc.tile_pool(name="xch", bufs=2))
        xqp = ctx.enter_context(tc.tile_pool(name="xq", bufs=2))
        qtp = ctx.enter_context(tc.tile_pool(name="qt", bufs=2))
        mtp = ctx.enter_context(tc.tile_pool(name="mt", bufs=2))
        ptp = ctx.enter_context(tc.tile_pool(name="pt", bufs=3))
        rbp = ctx.enter_context(tc.tile_pool(name="rb", bufs=2))
        ytp = ctx.enter_context(tc.tile_pool(name="yt", bufs=2))
        ostp = ctx.enter_context(tc.tile_pool(name="ost", bufs=2))
        psg = ctx.enter_context(tc.tile_pool(name="psg", bufs=2, space="PSUM"))
        psy = ctx.enter_context(tc.tile_pool(name="psy", bufs=2, space="PSUM"))
        psm = ctx.enter_context(tc.tile_pool(name="psm", bufs=2, space="PSUM"))

        # weights + first x chunk interleaved so the first matmuls start early
        wt = singles.tile([P, 6, WALL], f16)
        xch0 = xchp.tile([P, 6, BCW], f16, tag="xch", name="xch0")
        for c in range(6):
            rows = slice(P * c, P * c + P)
            nc.sync.dma_start(wt[:, c], wall[rows, :])
            nc.sync.dma_start(xch0[:, c], xt16[rows, 0:BCW])
        wpj_t = []
        for h in range(NH):
            w1 = singles.tile([D, C], f16, tag=f"wpj{h}", name=f"wpj{h}")
            nc.sync.dma_start(w1, wpj16[h])
            wpj_t.append(w1)
        bq_t = singles.tile([P, 2], f32)
        nc.sync.dma_start(bq_t, bq[:, :])

        ones_f = singles.tile([1, 64], f32)
        nc.vector.memset(ones_f, 1.0)
        ones64 = singles.tile([1, 64], f32r)
        nc.vector.tensor_copy(ones64, ones_f)
        bln16 = singles.tile([P, 1], f32)   # exp bias -ln(16): fp16 headroom
        nc.vector.memset(bln16, -2.772588722239781)
        # head-select matrices: e3[:, 64h:64h+64] has row 32h all-ones, rest 0;
        # bc_h = e3_h^T @ rdens broadcasts head h's denominator row (base-0
        # rhs; AP partition starts are restricted to 0/32/64, hence the 32h
        # row placement)
        e3f = singles.tile([65, NH * 64], f32)
        nc.vector.memset(e3f, 0.0)
        for h in range(NH):
            nc.vector.memset(e3f[32 * h:32 * h + 1, 64 * h:64 * h + 64], 1.0)
        e3 = singles.tile([65, NH * 64], f32r)
        nc.vector.tensor_copy(e3, e3f)

        # K^T in [d, k] layout: heads 0,1 stacked on 128 partitions; head 2 alone
        kt01 = singles.tile([P, T], f16, tag="kt01", name="kt01")
        kt2 = singles.tile([D, T], f16, tag="kt2", name="kt2")
        # V in [k, d] layout, per k-block: [v0(64) 1 | v1(64) 1 | v2(64) 1]
        vp3 = singles.tile([P, NKB, NH * 65], f16, tag="vp3", name="vp3")
        vp3_ones = vp3.rearrange("p b (h c) -> p b h c", c=65)[:, :, :, 64:65]
        nc.vector.memset(vp3_ones, 1.0)

        # ---- Phase 1: K^T and V from x^T, in 512-col compute chunks ----
        for big in range(NBC):
            if big == 0:
                xch = xch0
            else:
                xch = xchp.tile([P, 6, BCW], f16, tag="xch", name=f"xch{big}")
                nc.sync.dma_start(
                    xch, xt16[:, BCW * big:BCW * (big + 1)].rearrange(
                        "(c p) t -> p c t", p=P))
            for sub in range(2):
                cs = slice(512 * sub, 512 * (sub + 1))
                cols = slice(BCW * big + 512 * sub, BCW * big + 512 * (sub + 1))
                ps01 = psg.tile([P, 512], f32, tag="sg", name="ps01")
                for c in range(6):
                    nc.tensor.matmul(ps01, wt[:, c, WK01:WK01 + 128],
                                     xch[:, c, cs], start=(c == 0), stop=(c == 5))
                nc.scalar.activation(kt01[:, cols], ps01, CPY)
                ps2 = psg.tile([D, 512], f32, tag="sg", name="ps2")
                for c in range(6):
                    nc.tensor.matmul(ps2, wt[:, c, WK2:WK2 + 64],
                                     xch[:, c, cs], start=(c == 0), stop=(c == 5))
                nc.vector.tensor_copy(kt2[:, cols], ps2)
                for i in range(4):
                    kb = 8 * big + 4 * sub + i
                    psv = psy.tile([P, NH * D], f32, tag="yacc", name="psv")
                    for c in range(6):
                        nc.tensor.matmul(
                            psv, xch[:, c, 512 * sub + P * i:512 * sub + P * (i + 1)],
                            wt[:, c, WV3:WV3 + 192], start=(c == 0), stop=(c == 5))
                    nc.vector.tensor_copy(
                        vp3[:, kb].rearrange("p (h c) -> p h c", c=65)[:, :, 0:64],
                        psv.rearrange("p (h c) -> p h c", c=64))

        # ---- Phase 2: per q-slot: Q^T, attention, projection ----
        # Slot j's normalize+projection is deferred into slot j+1's stream
        # (between heads 0 and 1) so the PE never waits on the DVE reciprocal.
        def emit_qkv(j):
            xq = xqp.tile([P, 6, QC], f16, tag="xq", name="xq")
            nc.sync.dma_start(
                xq, xtq16[:, QC * j:QC * (j + 1)].rearrange("(c p) q -> p c q", p=P))
            mt = mtp.tile([P, 1024], f16, tag="mt", name="mt")
            nc.sync.dma_start(mt, mask[j])
            psq01 = psm.tile([P, QC], f32, tag="psm", name="psq01")
            for c in range(6):
                nc.tensor.matmul(psq01, wt[:, c, WQ01:WQ01 + 128], xq[:, c],
                                 start=(c == 0), stop=(c == 5))
            qt01 = qtp.tile([P, QC], f16, tag="qt01", name="qt01")
            nc.vector.tensor_scalar(qt01, psq01, bq_t[:, 0:1], None, ADD)
            psq2 = psm.tile([D, QC], f32, tag="psm", name="psq2")
            for c in range(6):
                nc.tensor.matmul(psq2, wt[:, c, WQ2:WQ2 + 64], xq[:, c],
                                 start=(c == 0), stop=(c == 5))
            qt2 = qtp.tile([D, QC], f16, tag="qt2", name="qt2")
            nc.vector.tensor_scalar(qt2, psq2, bq_t[0:D, 1:2], None, ADD)
            return [qt01[0:D], qt01[D:P], qt2], mt

        def emit_attn_head(j, h, qts, mt, dens):
            kts = [kt01[0:D], kt01[D:P], kt2]
            yacc = psy.tile([65, QC], f32, tag="yacc", name="yacc")

            def s_group(g):
                sg = psg.tile([P, 1024], f32, tag="sg", name="sg")
                for i in range(4):
                    kb = 4 * g + i
                    nc.tensor.matmul(sg[:, QC * i:QC * (i + 1)],
                                     kts[h][:, P * kb:P * (kb + 1)],
                                     qts[h], start=True, stop=True)
                return sg

            # software pipeline: issue S(g+1) before PV(g) so the PE has
            # work while ACT runs exp(g)
            sg_cur = s_group(0)
            for g in range(j + 1):
                sg_next = s_group(g + 1) if g < j else None
                pt = ptp.tile([P, 1024], f16, tag="pt", name="pt")
                nc.scalar.activation(pt, sg_cur, EXP, scale=0.125,
                                     bias=bln16[:, 0:1])
                if g == j:
                    nc.vector.tensor_mul(pt, pt, mt)
                for i in range(4):
                    kb = 4 * g + i
                    nc.tensor.matmul(yacc, vp3[:, kb, 65 * h:65 * h + 65],
                                     pt[:, QC * i:QC * (i + 1)],
                                     start=(g == 0 and i == 0),
                                     stop=(g == j and i == 3))
                sg_cur = sg_next
            nc.vector.tensor_copy(dens[32 * h:32 * h + 1], yacc[64:65])
            ytu = ytp.tile([64, QC], f16, tag=f"ytu{h}", name=f"ytu{h}")
            nc.vector.tensor_copy(ytu, yacc[0:64])
            return ytu

        def emit_norm_proj(j, dens, ytu_t):
            # one batched reciprocal for all 3 heads (rows 0/32/64 of dens;
            # filler lanes are 1.0 so the selector matmul stays finite)
            rdens32 = rbp.tile([65, QC], f32, tag="rdens32", name="rdens32")
            nc.vector.reciprocal(rdens32, dens)
            rdens = rbp.tile([65, QC], f32r, tag="rdens", name="rdens")
            nc.vector.tensor_copy(rdens, rdens32)
            yt_t = []
            for h in range(NH):
                bc = psm.tile([64, QC], f32, tag="psm", name="bc")
                nc.tensor.matmul(bc, e3[:, 64 * h:64 * h + 64], rdens,
                                 start=True, stop=True)
                yt = ytp.tile([64, QC], f16, tag=f"yt{h}", name=f"yt{h}")
                nc.vector.tensor_mul(yt, ytu_t[h], bc)
                yt_t.append(yt)
            ost = ostp.tile([P, 2, C], f32, tag="ost", name="ost")
            for qb in range(2):
                for (n0, nw) in [(0, 512), (512, 256)]:
                    pp = psm.tile([P, nw], f32, tag="psm", name="pp")
                    for h in range(NH):
                        nc.tensor.matmul(pp, yt_t[h][:, P * qb:P * (qb + 1)],
                                         wpj_t[h][:, n0:n0 + nw],
                                         start=(h == 0), stop=(h == NH - 1))
                    nc.vector.tensor_copy(ost[:, qb, n0:n0 + nw], pp)
            for qb in range(2):
                nc.sync.dma_start(
                    out[QC * j + P * qb:QC * j + P * (qb + 1), :], ost[:, qb])

        pending = None   # (j, dens, ytu_t) awaiting normalize+projection
        for j in range(NSLOT):
            qts, mt = emit_qkv(j)
            dens = rbp.tile([65, QC], f32, tag="dens", name="dens")
            nc.vector.memset(dens, 1.0)
            ytu_t = []
            for h in range(NH):
                ytu_t.append(emit_attn_head(j, h, qts, mt, dens))
                if h == 0 and pending is not None:
                    emit_norm_proj(*pending)
                    pending = None
            pending = (j, dens, ytu_t)
        emit_norm_proj(*pending)

    split_multi_waits(nc)
    return nc


def make_in_maps(x, W_qkv, b_qkv, W_proj):
    """Shard the full inputs into the 8 per-core input maps."""
    xT = np.ascontiguousarray(x.reshape(T, C).T).astype(np.float32)
    xT16 = xT.astype(np.float16)

    kk = np.arange(P)
    qq = np.arange(QC)
    in_maps = []
    for core in range(8):
        hg, s = core // 2, core % 2
        heads = [3 * hg + i for i in range(NH)]
        wk = [W_qkv[:, C + 64 * h:C + 64 * h + 64] for h in heads]
        wv = [W_qkv[:, 2 * C + 64 * h:2 * C + 64 * h + 64] for h in heads]
        wq = [W_qkv[:, 64 * h:64 * h + 64] for h in heads]
        wall_c = np.concatenate(
            wk[0:2] + [wk[2]] + wv + wq[0:2] + [wq[2]], axis=1).astype(np.float16)
        wpj_c = np.stack([W_proj[64 * h:64 * h + 64, :] for h in heads]
                         ).astype(np.float16)

        bq_c = np.zeros((P, 2), np.float32)
        bq_c[0:64, 0] = b_qkv[64 * heads[0]:64 * heads[0] + 64]
        bq_c[64:P, 0] = b_qkv[64 * heads[1]:64 * heads[1] + 64]
        bq_c[0:64, 1] = b_qkv[64 * heads[2]:64 * heads[2] + 64]

        qcols = np.concatenate(
            [np.arange(QC * (2 * j + s), QC * (2 * j + s) + QC)
             for j in range(NSLOT)])
        xtq_16 = np.ascontiguousarray(xT16[:, qcols])

        mask_c = np.zeros((NSLOT, P, 1024), np.float32)
        for j in range(NSLOT):
            q0 = QC * (2 * j + s)
            for i in range(4):
                k0 = P * (4 * j + i)
                mask_c[j, :, QC * i:QC * (i + 1)] = (
                    (k0 + kk[:, None]) <= (q0 + qq[None, :]))

        in_maps.append({
            "xt16": xT16, "xtq16": xtq_16, "wall": wall_c,
            "wpj16": wpj_c, "bq": bq_c, "mask": mask_c.astype(np.float16),
        })
    return in_maps


def unshard(results, b_qkv, W_proj, b_proj):
    out = np.zeros((T, C), np.float64)
    for core in range(8):
        s = core % 2
        r = results[core]["out"].astype(np.float64)
        for j in range(NSLOT):
            g0 = QC * (2 * j + s)
            out[g0:g0 + QC] += r[QC * j:QC * (j + 1)]
    # V-bias folded here: (y + b_v) @ W_proj = y @ W_proj + b_v @ W_proj
    b_eff = b_proj.astype(np.float64) + (
        b_qkv[2 * C:3 * C].astype(np.float64) @ W_proj.astype(np.float64))
    out += b_eff
    return out.astype(np.float32).reshape(1, T, C)


_last_result = {}


def kernel(x, mask, W_qkv, b_qkv, W_proj, b_proj):
    from concourse.bass_utils import run_bass_kernel_spmd
    x = np.asarray(x, np.float32)
    W_qkv = np.asarray(W_qkv, np.float32)
    b_qkv = np.asarray(b_qkv, np.float32)
    W_proj = np.asarray(W_proj, np.float32)
    b_proj = np.asarray(b_proj, np.float32)

    if "nc" not in _nc_cache:
        _nc_cache["nc"] = build_nc()
    nc = _nc_cache["nc"]
    in_maps = make_in_maps(x, W_qkv, b_qkv, W_proj)
    import os
    kwargs = {}
    if os.environ.get("BASS_KERNEL_TRACE"):
        kwargs = dict(trace=True, trace_cores=list(range(8)))
    res = run_bass_kernel_spmd(nc, in_maps, core_ids=list(range(8)), **kwargs)
    _last_result["res"] = res
    return unshard([r for r in res.results], b_qkv, W_proj, b_proj)


# revision 23
# speedup vs baseline: 1.4466x; 1.1677x over previous
"""Causal self-attention (B=1, T=4096, C=768, H=12, D=64) on 8 TRN2 NeuronCores.

Sharding: 8 cores = 4 head-groups (3 heads each) x 2 sequence-groups.
Core c: heads [3*hg, 3*hg+2] where hg=c//2; handles q-chunks of 256 rows,
global chunk g = 2*j + s (s=c%2, j=0..7) -- interleaving balances the causal
triangle so every core runs an identical instruction stream (SPMD), with the
boundary masks supplied as per-core data.

Every matmul runs in fp16 single-pass (fp32 PSUM accumulation; all operands
O(1) so fp16 range is ample).  Flash-style attention in S^T = K^T q
orientation with K=64 contraction (matmul cost depends only on the moving
size, so no 128-partition packing is needed for lhsT/rhs).  V is produced
directly in [k, d] layout (lhsT = x^T chunk, rhs = W_v), avoiding PE
transposes; a per-head ones column interleaved in the V tile makes the
softmax denominator fall out of the same PV matmuls.  K-bias is dropped
(softmax shift invariance, exact); V-bias is folded into b_proj on the host
(linearity, exact).  Causality = block skipping + 0/1 boundary masks.
The attention inner loop is ACT(exp)-paced, so each slot's normalize +
projection is emitted inside the NEXT slot's attention stream to keep the
PE queue fed.  Host sums the 4 head-group partial projections.
"""
import numpy as np

T, C, H, D = 4096, 768, 12, 64
NH = 3          # heads per core
QC = 256        # q rows per slot
P = 128
NKB = T // P    # 32 k-blocks
NBC = 4         # phase-1 big DMA chunks
BCW = T // NBC  # 1024 cols per big chunk
NSLOT = T // (2 * QC)   # 8 q-slots per core
TQ = NSLOT * QC         # 2048 q rows per core
# fused weight-tensor column layout: [wk01 | wk2 | wv3 | wq01 | wq2]
WK01, WK2, WV3, WQ01, WQ2 = 0, 128, 192, 384, 512
WALL = 576

_nc_cache = {}


def split_multi_waits(nc):
    """Walrus here accepts only one sync wait per instruction: hoist extras
    onto standalone InstEventSemaphore instructions on the same engine."""
    import concourse.mybir as mybir
    n_split = 0
    for f in nc.m.functions:
        for bb in f.blocks:
            new_insts = []
            for inst in bb.instructions:
                si = inst.sync_info
                if si is not None and len(si.on_wait) > 1:
                    for w in si.on_wait[:-1]:
                        nop = mybir.InstEventSemaphore(
                            name=nc.get_next_instruction_name(), ins=[], outs=[])
                        nop.engine = inst.engine
                        nop.sync_info = mybir.SyncInfo(on_wait=[w], on_update=[])
                        nc.register_instruction(nop)
                        new_insts.append(nop)
                        n_split += 1
                    si.on_wait = si.on_wait[-1:]
                new_insts.append(inst)
            bb.instructions[:] = new_insts
    return n_split


def build_nc():
    import concourse.bass as bass
    import concourse.mybir as mybir
    import concourse.tile as tile
    from contextlib import ExitStack

    f32r = mybir.dt.float32r
    f32 = mybir.dt.float32
    f16 = mybir.dt.float16
    EXP = mybir.ActivationFunctionType.Exp
    CPY = mybir.ActivationFunctionType.Copy
    ADD = mybir.AluOpType.add

    nc = bass.Bass(trn_type="TRN2")
    xt16 = nc.dram_tensor("xt16", [C, T], f16, kind="ExternalInput")
    xtq16 = nc.dram_tensor("xtq16", [C, TQ], f16, kind="ExternalInput")
    wall = nc.dram_tensor("wall", [C, WALL], f16, kind="ExternalInput")
    wpj16 = nc.dram_tensor("wpj16", [NH, D, C], f16, kind="ExternalInput")
    bq = nc.dram_tensor("bq", [P, 2], f32, kind="ExternalInput")
    mask = nc.dram_tensor("mask", [NSLOT, P, 1024], f16, kind="ExternalInput")
    out = nc.dram_tensor("out", [TQ, C], f32, kind="ExternalOutput")

    with tile.TileContext(nc) as tc, ExitStack() as ctx:
        singles = ctx.enter_context(tc.tile_pool(name="singles", bufs=1))
        xchp = ctx.enter_context(tc.tile_pool(name="xch", bufs=2))
        xqp = ctx.enter_context(tc.tile_pool(name="xq", bufs=2))
        qtp = ctx.enter_context(tc.tile_pool(name="qt", bufs=2))
        mtp = ctx.enter_context(tc.tile_pool(name="mt", bufs=2))
        ptp = ctx.enter_context(tc.tile_pool(name="pt", bufs=3))
        rbp = ctx.enter_context(tc.tile_pool(name="rb", bufs=2))
        ytp = ctx.enter_context(tc.tile_pool(name="yt", bufs=2))
        ostp = ctx.enter_context(tc.tile_pool(name="ost", bufs=2))
        psg = ctx.enter_context(tc.tile_pool(name="psg", bufs=2, space="PSUM"))
        psy = ctx.enter_context(tc.tile_pool(name="psy", bufs=2, space="PSUM"))
        psm = ctx.enter_context(tc.tile_pool(name="psm", bufs=2, space="PSUM"))

        # weights + first x chunk interleaved so the first matmuls start early
        wt = singles.tile([P, 6, WALL], f16)
        xch0 = xchp.tile([P, 6, BCW], f16, tag="xch", name="xch0")
        for c in range(6):
            rows = slice(P * c, P * c + P)
            nc.sync.dma_start(wt[:, c], wall[rows, :])
            nc.sync.dma_start(xch0[:, c], xt16[rows, 0:BCW])
        wpj_t = []
        for h in range(NH):
            w1 = singles.tile([D, C], f16, tag=f"wpj{h}", name=f"wpj{h}")
            nc.sync.dma_start(w1, wpj16[h])
            wpj_t.append(w1)
        bq_t = singles.tile([P, 2], f32)
        nc.sync.dma_start(bq_t, bq[:, :])

        ones_f = singles.tile([1, 64], f32)
        nc.vector.memset(ones_f, 1.0)
        ones64 = singles.tile([1, 64], f32r)
        nc.vector.tensor_copy(ones64, ones_f)
        bln16 = singles.tile([P, 1], f32)   # exp bias -ln(16): fp16 headroom
        nc.vector.memset(bln16, -2.772588722239781)
        # head-select matrices: e3[:, 64h:64h+64] has row 32h all-ones, rest 0;
        # bc_h = e3_h^T @ rdens broadcasts head h's denominator row (base-0
        # rhs; AP partition starts are restricted to 0/32/64, hence the 32h
        # row placement)
        e3f = singles.tile([65, NH * 64], f32)
        nc.vector.memset(e3f, 0.0)
        for h in range(NH):
            nc.vector.memset(e3f[32 * h:32 * h + 1, 64 * h:64 * h + 64], 1.0)
        e3 = singles.tile([65, NH * 64], f32r)
        nc.vector.tensor_copy(e3, e3f)

        # K^T in [d, k] layout: heads 0,1 stacked on 128 partitions; head 2 alone
        kt01 = singles.tile([P, T], f16, tag="kt01", name="kt01")
        kt2 = singles.tile([D, T], f16, tag="kt2", name="kt2")
        # V in [k, d] layout, per k-block: [v0(64) 1 | v1(64) 1 | v2(64) 1]
        vp3 = singles.tile([P, NKB, NH * 65], f16, tag="vp3", name="vp3")
        vp3_ones = vp3.rearrange("p b (h c) -> p b h c", c=65)[:, :, :, 64:65]
        nc.vector.memset(vp3_ones, 1.0)

        # ---- Phase 1: K^T and V from x^T, as emission units ----
        # Chunk c (512 k-cols) must be resident before slot c's attention.
        # Chunk 0 is emitted upfront; chunks 1..7 are drained as PE filler
        # between attention PV groups (the attention inner loop is ACT-paced,
        # so this keeps the PE queue fed and the HAM clock warm).
        xch_tiles = {0: xch0}

        def dma_unit(big):
            def emit():
                xch = xchp.tile([P, 6, BCW], f16, tag="xch", name=f"xch{big}")
                nc.sync.dma_start(
                    xch, xt16[:, BCW * big:BCW * (big + 1)].rearrange(
                        "(c p) t -> p c t", p=P))
                xch_tiles[big] = xch
            return emit

        def k_unit(ch, woff, wlen, kt, nm):
            big, sub = ch // 2, ch % 2
            cols = slice(512 * ch, 512 * (ch + 1))

            def emit():
                xch = xch_tiles[big]
                ps = psm.tile([wlen, 512], f32, tag="psm", name=nm)
                for c in range(6):
                    nc.tensor.matmul(ps, wt[:, c, woff:woff + wlen],
                                     xch[:, c, 512 * sub:512 * (sub + 1)],
                                     start=(c == 0), stop=(c == 5))
                if nm == "ps01":
                    nc.scalar.activation(kt[:, cols], ps, CPY)
                else:
                    nc.vector.tensor_copy(kt[:, cols], ps)
            return emit

        def v_unit(ch, i):
            big, sub = ch // 2, ch % 2
            kb = 4 * ch + i

            def emit():
                xch = xch_tiles[big]
                psv = psm.tile([P, NH * D], f32, tag="psm", name="psv")
                for c in range(6):
                    nc.tensor.matmul(
                        psv, xch[:, c, 512 * sub + P * i:512 * sub + P * (i + 1)],
                        wt[:, c, WV3:WV3 + 192], start=(c == 0), stop=(c == 5))
                nc.vector.tensor_copy(
                    vp3[:, kb].rearrange("p (h c) -> p h c", c=65)[:, :, 0:64],
                    psv.rearrange("p (h c) -> p h c", c=64))
            return emit

        def chunk_units(ch):
            return ([k_unit(ch, WK01, 128, kt01, "ps01"),
                     k_unit(ch, WK2, 64, kt2, "ps2")] +
                    [v_unit(ch, i) for i in range(4)])

        dma_unit(1)()                      # prefetch second big x chunk
        for u in chunk_units(0):           # chunk 0 upfront (slot 0 needs it)
            u()

        filler = []
        for ch in range(1, 8):
            if ch == 4:
                filler.append(dma_unit(2))
            if ch == 6:
                filler.append(dma_unit(3))
            filler.extend(chunk_units(ch))
        # FIFO index that must be drained by the end of each slot j
        # (chunk j+1 resident before slot j+1; DMA units ride along)
        need_by_end = []
        acc = 0
        for ch in range(1, 8):
            acc += 6 + (1 if ch in (4, 6) else 0)
            need_by_end.append(acc)
        need_by_end = need_by_end + [acc]   # j=7: nothing new
        fed = [0]

        def feed(j, g, G):
            lo = need_by_end[j - 1] if j > 0 else 0
            hi = need_by_end[j] if j < len(need_by_end) else len(filler)
            target = lo + ((hi - lo) * (g + 1) + G - 1) // G
            while fed[0] < min(target, len(filler)):
                filler[fed[0]]()
                fed[0] += 1

        # ---- Phase 2: per q-slot: Q^T, attention, projection ----
        # Slot j's normalize+projection is deferred into slot j+1's stream
        # (between heads 0 and 1) so the PE never waits on the DVE reciprocal.
        def emit_qkv(j):
            xq = xqp.tile([P, 6, QC], f16, tag="xq", name="xq")
            nc.sync.dma_start(
                xq, xtq16[:, QC * j:QC * (j + 1)].rearrange("(c p) q -> p c q", p=P))
            mt = mtp.tile([P, 1024], f16, tag="mt", name="mt")
            nc.sync.dma_start(mt, mask[j])
            psq01 = psm.tile([P, QC], f32, tag="psm", name="psq01")
            for c in range(6):
                nc.tensor.matmul(psq01, wt[:, c, WQ01:WQ01 + 128], xq[:, c],
                                 start=(c == 0), stop=(c == 5))
            qt01 = qtp.tile([P, QC], f16, tag="qt01", name="qt01")
            nc.vector.tensor_scalar(qt01, psq01, bq_t[:, 0:1], None, ADD)
            psq2 = psm.tile([D, QC], f32, tag="psm", name="psq2")
            for c in range(6):
                nc.tensor.matmul(psq2, wt[:, c, WQ2:WQ2 + 64], xq[:, c],
                                 start=(c == 0), stop=(c == 5))
            qt2 = qtp.tile([D, QC], f16, tag="qt2", name="qt2")
            nc.vector.tensor_scalar(qt2, psq2, bq_t[0:D, 1:2], None, ADD)
            return [qt01[0:D], qt01[D:P], qt2], mt

        def emit_attn_head(j, h, qts, mt, dens, g_base):
            kts = [kt01[0:D], kt01[D:P], kt2]
            yacc = psy.tile([65, QC], f32, tag="yacc", name="yacc")
            G = NH * (j + 1)

            def s_group(g):
                sg = psg.tile([P, 1024], f32, tag="sg", name="sg")
                for i in range(4):
                    kb = 4 * g + i
                    nc.tensor.matmul(sg[:, QC * i:QC * (i + 1)],
                                     kts[h][:, P * kb:P * (kb + 1)],
                                     qts[h], start=True, stop=True)
                return sg

            # software pipeline: issue S(g+1) before PV(g) so the PE has
            # work while ACT runs exp(g)
            sg_cur = s_group(0)
            for g in range(j + 1):
                sg_next = s_group(g + 1) if g < j else None
                pt = ptp.tile([P, 1024], f16, tag="pt", name="pt")
                nc.scalar.activation(pt, sg_cur, EXP, scale=0.125,
                                     bias=bln16[:, 0:1])
                if g == j:
                    nc.vector.tensor_mul(pt, pt, mt)
                for i in range(4):
                    kb = 4 * g + i
                    nc.tensor.matmul(yacc, vp3[:, kb, 65 * h:65 * h + 65],
                                     pt[:, QC * i:QC * (i + 1)],
                                     start=(g == 0 and i == 0),
                                     stop=(g == j and i == 3))
                feed(j, g_base + g, G)
                sg_cur = sg_next
            nc.vector.tensor_copy(dens[32 * h:32 * h + 1], yacc[64:65])
            ytu = ytp.tile([64, QC], f16, tag=f"ytu{h}", name=f"ytu{h}")
            nc.vector.tensor_copy(ytu, yacc[0:64])
            return ytu

        def emit_norm_proj(j, dens, ytu_t):
            # one batched reciprocal for all 3 heads (rows 0/32/64 of dens;
            # filler lanes are 1.0 so the selector matmul stays finite)
            rdens32 = rbp.tile([65, QC], f32, tag="rdens32", name="rdens32")
            nc.vector.reciprocal(rdens32, dens)
            rdens = rbp.tile([65, QC], f32r, tag="rdens", name="rdens")
            nc.vector.tensor_copy(rdens, rdens32)
            yt_t = []
            for h in range(NH):
                bc = psm.tile([64, QC], f32, tag="psm", name="bc")
                nc.tensor.matmul(bc, e3[:, 64 * h:64 * h + 64], rdens,
                                 start=True, stop=True)
                yt = ytp.tile([64, QC], f16, tag=f"yt{h}", name=f"yt{h}")
                nc.vector.tensor_mul(yt, ytu_t[h], bc)
                yt_t.append(yt)
            ost = ostp.tile([P, 2, C], f32, tag="ost", name="ost")
            for qb in range(2):
                for (n0, nw) in [(0, 512), (512, 256)]:
                    pp = psm.tile([P, nw], f32, tag="psm", name="pp")
                    for h in range(NH):
                        nc.tensor.matmul(pp, yt_t[h][:, P * qb:P * (qb + 1)],
                                         wpj_t[h][:, n0:n0 + nw],
                                         start=(h == 0), stop=(h == NH - 1))
                    nc.vector.tensor_copy(ost[:, qb, n0:n0 + nw], pp)
            for qb in range(2):
                nc.sync.dma_start(
                    out[QC * j + P * qb:QC * j + P * (qb + 1), :], ost[:, qb])

        pending = None   # (j, dens, ytu_t) awaiting normalize+projection
        for j in range(NSLOT):
            qts, mt = emit_qkv(j)
            dens = rbp.tile([65, QC], f32, tag="dens", name="dens")
            nc.vector.memset(dens, 1.0)
            ytu_t = []
            for h in range(NH):
                ytu_t.append(emit_attn_head(j, h, qts, mt, dens, h * (j + 1)))
                if h == 0 and pending is not None:
                    emit_norm_proj(*pending)
                    pending = None
            pending = (j, dens, ytu_t)
        emit_norm_proj(*pending)

    split_multi_waits(nc)
    return nc


def make_in_maps(x, W_qkv, b_qkv, W_proj):
    """Shard the full inputs into the 8 per-core input maps."""
    xT = np.ascontiguousarray(x.reshape(T, C).T).astype(np.float32)
    xT16 = xT.astype(np.float16)

    kk = np.arange(P)
    qq = np.arange(QC)
    in_maps = []
    for core in range(8):
        hg, s = core // 2, core % 2
        heads = [3 * hg + i for i in range(NH)]
        wk = [W_qkv[:, C + 64 * h:C + 64 * h + 64] for h in heads]
        wv = [W_qkv[:, 2 * C + 64 * h:2 * C + 64 * h + 64] for h in heads]
        wq = [W_qkv[:, 64 * h:64 * h + 64] for h in heads]
        wall_c = np.concatenate(
            wk[0:2] + [wk[2]] + wv + wq[0:2] + [wq[2]], axis=1).astype(np.float16)
        wpj_c = np.stack([W_proj[64 * h:64 * h + 64, :] for h in heads]
                         ).astype(np.float16)

        bq_c = np.zeros((P, 2), np.float32)
        bq_c[0:64, 0] = b_qkv[64 * heads[0]:64 * heads[0] + 64]
        bq_c[64:P, 0] = b_qkv[64 * heads[1]:64 * heads[1] + 64]
        bq_c[0:64, 1] = b_qkv[64 * heads[2]:64 * heads[2] + 64]

        qcols = np.concatenate(
            [np.arange(QC * (2 * j + s), QC * (2 * j + s) + QC)
             for j in range(NSLOT)])
        xtq_16 = np.ascontiguousarray(xT16[:, qcols])

        mask_c = np.zeros((NSLOT, P, 1024), np.float32)
        for j in range(NSLOT):
            q0 = QC * (2 * j + s)
            for i in range(4):
                k0 = P * (4 * j + i)
                mask_c[j, :, QC * i:QC * (i + 1)] = (
                    (k0 + kk[:, None]) <= (q0 + qq[None, :]))

        in_maps.append({
            "xt16": xT16, "xtq16": xtq_16, "wall": wall_c,
            "wpj16": wpj_c, "bq": bq_c, "mask": mask_c.astype(np.float16),
        })
    return in_maps


def unshard(results, b_qkv, W_proj, b_proj):
    out = np.zeros((T, C), np.float64)
    for core in range(8):
        s = core % 2
        r = results[core]["out"].astype(np.float64)
        for j in range(NSLOT):
            g0 = QC * (2 * j + s)
            out[g0:g0 + QC] += r[QC * j:QC * (j + 1)]
    # V-bias folded here: (y + b_v) @ W_proj = y @ W_proj + b_v @ W_proj
    b_eff = b_proj.astype(np.float64) + (
        b_qkv[2 * C:3 * C].astype(np.float64) @ W_proj.astype(np.float64))
    out += b_eff
    return out.astype(np.float32).reshape(1, T, C)


_last_result = {}


def kernel(x, mask, W_qkv, b_qkv, W_proj, b_proj):
    from concourse.bass_utils import run_bass_kernel_spmd
    x = np.asarray(x, np.float32)
    W_qkv = np.asarray(W_qkv, np.float32)
    b_qkv = np.asarray(b_qkv, np.float32)
    W_proj = np.asarray(W_proj, np.float32)
    b_proj = np.asarray(b_proj, np.float32)

    if "nc" not in _nc_cache:
        _nc_cache["nc"] = build_nc()
    nc = _nc_cache["nc"]
    in_maps = make_in_maps(x, W_qkv, b_qkv, W_proj)
    import os
    kwargs = {}
    if os.environ.get("BASS_KERNEL_TRACE"):
        kwargs = dict(trace=True, trace_cores=list(range(8)))
    res = run_bass_kernel_spmd(nc, in_maps, core_ids=list(range(8)), **kwargs)
    _last_result["res"] = res
    return unshard([r for r in res.results], b_qkv, W_proj, b_proj)


# revision 27
# speedup vs baseline: 1.4926x; 1.0319x over previous
"""Causal self-attention (B=1, T=4096, C=768, H=12, D=64) on 8 TRN2 NeuronCores.

Sharding: 8 cores = 4 head-groups (3 heads each) x 2 sequence-groups.
Core c: heads [3*hg, 3*hg+2] where hg=c//2; handles q-chunks of 256 rows,
global chunk g = 2*j + s (s=c%2, j=0..7) -- interleaving balances the causal
triangle so every core runs an identical instruction stream (SPMD), with the
boundary masks supplied as per-core data.

Every matmul runs in fp16 single-pass (fp32 PSUM accumulation; all operands
O(1) so fp16 range is ample).  Flash-style attention in S^T = K^T q
orientation with K=64 contraction (matmul cost depends only on the moving
size, so no 128-partition packing is needed for lhsT/rhs).  V is produced
directly in [k, d] layout (lhsT = x^T chunk, rhs = W_v), avoiding PE
transposes; a per-head ones column interleaved in the V tile makes the
softmax denominator fall out of the same PV matmuls.  K-bias is dropped
(softmax shift invariance, exact); V-bias is folded into b_proj on the host
(linearity, exact).  Causality = block skipping + 0/1 boundary masks.
The attention inner loop is ACT(exp)-paced, so each slot's normalize +
projection is emitted inside the NEXT slot's attention stream to keep the
PE queue fed.  Host sums the 4 head-group partial projections.
"""
import numpy as np

T, C, H, D = 4096, 768, 12, 64
NH = 3          # heads per core
QC = 256        # q rows per slot
P = 128
NKB = T // P    # 32 k-blocks
NBC = 4         # phase-1 big DMA chunks
BCW = T // NBC  # 1024 cols per big chunk
NSLOT = T // (2 * QC)   # 8 q-slots per core
TQ = NSLOT * QC         # 2048 q rows per core
# fused weight-tensor column layout: [wk01 | wk2 | wv3 | wq01 | wq2]
WK01, WK2, WV3, WQ01, WQ2 = 0, 128, 192, 384, 512
WALL = 576

_nc_cache = {}


def split_multi_waits(nc):
    """Walrus here accepts only one sync wait per instruction: hoist extras
    onto standalone InstEventSemaphore instructions on the same engine."""
    import concourse.mybir as mybir
    n_split = 0
    for f in nc.m.functions:
        for bb in f.blocks:
            new_insts = []
            for inst in bb.instructions:
                si = inst.sync_info
                if si is not None and len(si.on_wait) > 1:
                    for w in si.on_wait[:-1]:
                        nop = mybir.InstEventSemaphore(
                            name=nc.get_next_instruction_name(), ins=[], outs=[])
                        nop.engine = inst.engine
                        nop.sync_info = mybir.SyncInfo(on_wait=[w], on_update=[])
                        nc.register_instruction(nop)
                        new_insts.append(nop)
                        n_split += 1
                    si.on_wait = si.on_wait[-1:]
                new_insts.append(inst)
            bb.instructions[:] = new_insts
    return n_split


def build_nc():
    import concourse.bass as bass
    import concourse.mybir as mybir
    import concourse.tile as tile
    from contextlib import ExitStack

    f32r = mybir.dt.float32r
    f32 = mybir.dt.float32
    f16 = mybir.dt.float16
    EXP = mybir.ActivationFunctionType.Exp
    CPY = mybir.ActivationFunctionType.Copy
    ADD = mybir.AluOpType.add

    nc = bass.Bass(trn_type="TRN2")
    xt16 = nc.dram_tensor("xt16", [C, T], f16, kind="ExternalInput")
    xtq16 = nc.dram_tensor("xtq16", [C, TQ], f16, kind="ExternalInput")
    wall = nc.dram_tensor("wall", [C, WALL], f16, kind="ExternalInput")
    wpj16 = nc.dram_tensor("wpj16", [NH, D, C], f16, kind="ExternalInput")
    bq = nc.dram_tensor("bq", [P, 2], f32, kind="ExternalInput")
    mask = nc.dram_tensor("mask", [NSLOT, P, 1024], f16, kind="ExternalInput")
    out = nc.dram_tensor("out", [TQ, C], f32, kind="ExternalOutput")

    with tile.TileContext(nc) as tc, ExitStack() as ctx:
        singles = ctx.enter_context(tc.tile_pool(name="singles", bufs=1))
        xchp = ctx.enter_context(tc.tile_pool(name="xch", bufs=2))
        xqp = ctx.enter_context(tc.tile_pool(name="xq", bufs=2))
        qtp = ctx.enter_context(tc.tile_pool(name="qt", bufs=2))
        mtp = ctx.enter_context(tc.tile_pool(name="mt", bufs=2))
        ptp = ctx.enter_context(tc.tile_pool(name="pt", bufs=3))
        rbp = ctx.enter_context(tc.tile_pool(name="rb", bufs=2))
        ytp = ctx.enter_context(tc.tile_pool(name="yt", bufs=2))
        ostp = ctx.enter_context(tc.tile_pool(name="ost", bufs=2))
        psg = ctx.enter_context(tc.tile_pool(name="psg", bufs=2, space="PSUM"))
        psy = ctx.enter_context(tc.tile_pool(name="psy", bufs=2, space="PSUM"))
        psm = ctx.enter_context(tc.tile_pool(name="psm", bufs=2, space="PSUM"))

        # weights + first x chunk interleaved so the first matmuls start early
        wt = singles.tile([P, 6, WALL], f16)
        xch0 = xchp.tile([P, 6, BCW], f16, tag="xch", name="xch0")
        for c in range(6):
            rows = slice(P * c, P * c + P)
            nc.sync.dma_start(wt[:, c], wall[rows, :])
            nc.sync.dma_start(xch0[:, c], xt16[rows, 0:BCW])
        wpj_t = []
        for h in range(NH):
            w1 = singles.tile([D, C], f16, tag=f"wpj{h}", name=f"wpj{h}")
            nc.sync.dma_start(w1, wpj16[h])
            wpj_t.append(w1)
        bq_t = singles.tile([P, 2], f32)
        nc.sync.dma_start(bq_t, bq[:, :])

        ones_f = singles.tile([1, 64], f32)
        nc.vector.memset(ones_f, 1.0)
        ones64 = singles.tile([1, 64], f32r)
        nc.vector.tensor_copy(ones64, ones_f)
        bln16 = singles.tile([P, 1], f32)   # exp bias -ln(16): fp16 headroom
        nc.vector.memset(bln16, -2.772588722239781)
        # head-select matrices: e3[:, 64h:64h+64] has row 32h all-ones, rest 0;
        # bc_h = e3_h^T @ rdens broadcasts head h's denominator row (base-0
        # rhs; AP partition starts are restricted to 0/32/64, hence the 32h
        # row placement)
        e3f = singles.tile([65, NH * 64], f32)
        nc.vector.memset(e3f, 0.0)
        for h in range(NH):
            nc.vector.memset(e3f[32 * h:32 * h + 1, 64 * h:64 * h + 64], 1.0)
        e3 = singles.tile([65, NH * 64], f32r)
        nc.vector.tensor_copy(e3, e3f)

        # K^T in [d, k] layout: heads 0,1 stacked on 128 partitions; head 2 alone
        kt01 = singles.tile([P, T], f16, tag="kt01", name="kt01")
        kt2 = singles.tile([D, T], f16, tag="kt2", name="kt2")
        # V in [k, d] layout, per k-block: [v0(64) 1 | v1(64) 1 | v2(64) 1]
        vp3 = singles.tile([P, NKB, NH * 65], f16, tag="vp3", name="vp3")
        vp3_ones = vp3.rearrange("p b (h c) -> p b h c", c=65)[:, :, :, 64:65]
        nc.vector.memset(vp3_ones, 1.0)

        # ---- Phase 1: K^T and V from x^T, as emission units ----
        # Chunk c (512 k-cols) must be resident before slot c's attention.
        # Chunk 0 is emitted upfront; chunks 1..7 are drained as PE filler
        # between attention PV groups (the attention inner loop is ACT-paced,
        # so this keeps the PE queue fed and the HAM clock warm).
        xch_tiles = {0: xch0}

        def dma_unit(big):
            def emit():
                xch = xchp.tile([P, 6, BCW], f16, tag="xch", name=f"xch{big}")
                nc.sync.dma_start(
                    xch, xt16[:, BCW * big:BCW * (big + 1)].rearrange(
                        "(c p) t -> p c t", p=P))
                xch_tiles[big] = xch
            return emit

        def k_unit(ch, woff, wlen, kt, nm):
            big, sub = ch // 2, ch % 2
            cols = slice(512 * ch, 512 * (ch + 1))

            def emit():
                xch = xch_tiles[big]
                ps = psm.tile([wlen, 512], f32, tag="psm", name=nm)
                for c in range(6):
                    nc.tensor.matmul(ps, wt[:, c, woff:woff + wlen],
                                     xch[:, c, 512 * sub:512 * (sub + 1)],
                                     start=(c == 0), stop=(c == 5))
                if nm == "ps01":
                    nc.scalar.activation(kt[:, cols], ps, CPY)
                else:
                    nc.vector.tensor_copy(kt[:, cols], ps)
            return emit

        def v_unit(ch, i):
            big, sub = ch // 2, ch % 2
            kb = 4 * ch + i

            def emit():
                xch = xch_tiles[big]
                psv = psm.tile([P, NH * D], f32, tag="psm", name="psv")
                for c in range(6):
                    nc.tensor.matmul(
                        psv, xch[:, c, 512 * sub + P * i:512 * sub + P * (i + 1)],
                        wt[:, c, WV3:WV3 + 192], start=(c == 0), stop=(c == 5))
                nc.vector.tensor_copy(
                    vp3[:, kb].rearrange("p (h c) -> p h c", c=65)[:, :, 0:64],
                    psv.rearrange("p (h c) -> p h c", c=64))
            return emit

        def chunk_units(ch):
            return ([k_unit(ch, WK01, 128, kt01, "ps01"),
                     k_unit(ch, WK2, 64, kt2, "ps2")] +
                    [v_unit(ch, i) for i in range(4)])

        for u in chunk_units(0):           # chunk 0 upfront (slot 0 needs it)
            u()

        filler = []
        need_by_end = []
        for ch in range(1, 8):
            if ch == 1:
                filler.append(dma_unit(1))
            if ch == 4:
                filler.append(dma_unit(2))
            if ch == 6:
                filler.append(dma_unit(3))
            filler.extend(chunk_units(ch))
            # chunk ch (plus its prefetch DMAs) drained by end of slot ch-1
            need_by_end.append(len(filler))
        need_by_end.append(len(filler))     # j=7: nothing new
        fed = [0]
        np_state = [None, 0]                # pending norm/proj units, popped

        def feed(j, g, G):
            lo = need_by_end[j - 1] if j > 0 else 0
            hi = need_by_end[j] if j < len(need_by_end) else len(filler)
            target = lo + ((hi - lo) * (g + 1) + G - 1) // G
            while fed[0] < min(target, len(filler)):
                filler[fed[0]]()
                fed[0] += 1
            np_units = np_state[0]
            if np_units is not None:
                nt = (len(np_units) * (g + 1) + G - 1) // G
                while np_state[1] < min(nt, len(np_units)):
                    np_units[np_state[1]]()
                    np_state[1] += 1

        # ---- Phase 2: per q-slot: Q^T, attention, projection ----
        # Slot j's normalize+projection is deferred into slot j+1's stream
        # (between heads 0 and 1) so the PE never waits on the DVE reciprocal.
        def emit_qkv(j):
            xq = xqp.tile([P, 6, QC], f16, tag="xq", name="xq")
            nc.sync.dma_start(
                xq, xtq16[:, QC * j:QC * (j + 1)].rearrange("(c p) q -> p c q", p=P))
            mt = mtp.tile([P, 1024], f16, tag="mt", name="mt")
            nc.sync.dma_start(mt, mask[j])
            psq01 = psm.tile([P, QC], f32, tag="psm", name="psq01")
            for c in range(6):
                nc.tensor.matmul(psq01, wt[:, c, WQ01:WQ01 + 128], xq[:, c],
                                 start=(c == 0), stop=(c == 5))
            qt01 = qtp.tile([P, QC], f16, tag="qt01", name="qt01")
            nc.vector.tensor_scalar(qt01, psq01, bq_t[:, 0:1], None, ADD)
            psq2 = psm.tile([D, QC], f32, tag="psm", name="psq2")
            for c in range(6):
                nc.tensor.matmul(psq2, wt[:, c, WQ2:WQ2 + 64], xq[:, c],
                                 start=(c == 0), stop=(c == 5))
            qt2 = qtp.tile([D, QC], f16, tag="qt2", name="qt2")
            nc.vector.tensor_scalar(qt2, psq2, bq_t[0:D, 1:2], None, ADD)
            return [qt01[0:D], qt01[D:P], qt2], mt

        def emit_attn_head(j, h, qts, mt, dens, g_base):
            kts = [kt01[0:D], kt01[D:P], kt2]
            yacc = psy.tile([65, QC], f32, tag="yacc", name="yacc")
            G = NH * (j + 1)

            def s_group(g):
                sg = psg.tile([P, 1024], f32, tag="sg", name="sg")
                for i in range(4):
                    kb = 4 * g + i
                    nc.tensor.matmul(sg[:, QC * i:QC * (i + 1)],
                                     kts[h][:, P * kb:P * (kb + 1)],
                                     qts[h], start=True, stop=True)
                return sg

            # software pipeline: issue S(g+1) before PV(g) so the PE has
            # work while ACT runs exp(g)
            sg_cur = s_group(0)
            for g in range(j + 1):
                sg_next = s_group(g + 1) if g < j else None
                pt = ptp.tile([P, 1024], f16, tag="pt", name="pt")
                nc.scalar.activation(pt, sg_cur, EXP, scale=0.125,
                                     bias=bln16[:, 0:1])
                if g == j:
                    nc.vector.tensor_mul(pt, pt, mt)
                for i in range(4):
                    kb = 4 * g + i
                    nc.tensor.matmul(yacc, vp3[:, kb, 65 * h:65 * h + 65],
                                     pt[:, QC * i:QC * (i + 1)],
                                     start=(g == 0 and i == 0),
                                     stop=(g == j and i == 3))
                feed(j, g_base + g, G)
                sg_cur = sg_next
            nc.vector.tensor_copy(dens[32 * h:32 * h + 1], yacc[64:65])
            ytu = ytp.tile([64, QC], f16, tag=f"ytu{h}", name=f"ytu{h}")
            nc.vector.tensor_copy(ytu, yacc[0:64])
            return ytu

        def norm_proj_units(j, dens, ytu_t):
            # one batched reciprocal for all 3 heads (rows 0/32/64 of dens;
            # filler lanes are 1.0 so the selector matmul stays finite);
            # returned as fine-grained units fed between attention groups
            yt_t = []
            ost_box = []

            def u_norm(h):
                def emit():
                    if h == 0:
                        rdens32 = rbp.tile([65, QC], f32, tag="rdens32",
                                           name="rdens32")
                        nc.vector.reciprocal(rdens32, dens)
                        rdens = rbp.tile([65, QC], f32r, tag="rdens",
                                         name="rdens")
                        nc.vector.tensor_copy(rdens, rdens32)
                        ost_box.append(rdens)
                    rdens = ost_box[0]
                    bc = psm.tile([64, QC], f32, tag="psm", name="bc")
                    nc.tensor.matmul(bc, e3[:, 64 * h:64 * h + 64], rdens,
                                     start=True, stop=True)
                    yt = ytp.tile([64, QC], f16, tag=f"yt{h}", name=f"yt{h}")
                    nc.vector.tensor_mul(yt, ytu_t[h], bc)
                    yt_t.append(yt)
                return emit

            def u_proj(qb, n0, nw, last):
                def emit():
                    if len(ost_box) < 2:
                        ost_box.append(ostp.tile([P, 2, C], f32, tag="ost",
                                                 name="ost"))
                    ost = ost_box[1]
                    pp = psm.tile([P, nw], f32, tag="psm", name="pp")
                    for h in range(NH):
                        nc.tensor.matmul(pp, yt_t[h][:, P * qb:P * (qb + 1)],
                                         wpj_t[h][:, n0:n0 + nw],
                                         start=(h == 0), stop=(h == NH - 1))
                    nc.vector.tensor_copy(ost[:, qb, n0:n0 + nw], pp)
                    if last:
                        for q in range(2):
                            nc.sync.dma_start(
                                out[QC * j + P * q:QC * j + P * (q + 1), :],
                                ost[:, q])
                return emit

            return ([u_norm(h) for h in range(NH)] +
                    [u_proj(0, 0, 512, False), u_proj(0, 512, 256, False),
                     u_proj(1, 0, 512, False), u_proj(1, 512, 256, True)])

        pending = None   # norm/proj units of the previous slot
        for j in range(NSLOT):
            qts, mt = emit_qkv(j)
            if j == 0:
                filler[fed[0]]()   # xch1 prefetch DMA, behind slot-0's DMAs
                fed[0] += 1
            dens = rbp.tile([65, QC], f32, tag="dens", name="dens")
            nc.vector.memset(dens, 1.0)
            np_state[0] = pending
            np_state[1] = 0
            ytu_t = []
            for h in range(NH):
                ytu_t.append(emit_attn_head(j, h, qts, mt, dens, h * (j + 1)))
            if pending is not None:
                for k in range(np_state[1], len(pending)):
                    pending[k]()
            np_state[0] = None
            pending = norm_proj_units(j, dens, ytu_t)
        for u in pending:
            u()

    split_multi_waits(nc)
    return nc


def make_in_maps(x, W_qkv, b_qkv, W_proj):
    """Shard the full inputs into the 8 per-core input maps."""
    xT = np.ascontiguousarray(x.reshape(T, C).T).astype(np.float32)
    xT16 = xT.astype(np.float16)

    kk = np.arange(P)
    qq = np.arange(QC)
    in_maps = []
    for core in range(8):
        hg, s = core // 2, core % 2
        heads = [3 * hg + i for i in range(NH)]
        wk = [W_qkv[:, C + 64 * h:C + 64 * h + 64] for h in heads]
        wv = [W_qkv[:, 2 * C + 64 * h:2 * C + 64 * h + 64] for h in heads]
        wq = [W_qkv[:, 64 * h:64 * h + 64] for h in heads]
        wall_c = np.concatenate(
            wk[0:2] + [wk[2]] + wv + wq[0:2] + [wq[2]], axis=1).astype(np.float16)
        wpj_c = np.stack([W_proj[64 * h:64 * h + 64, :] for h in heads]
                         ).astype(np.float16)

        bq_c = np.zeros((P, 2), np.float32)
        bq_c[0:64, 0] = b_qkv[64 * heads[0]:64 * heads[0] + 64]
        bq_c[64:P, 0] = b_qkv[64 * heads[1]:64 * heads[1] + 64]
        bq_c[0:64, 1] = b_qkv[64 * heads[2]:64 * heads[2] + 64]

        qcols = np.concatenate(
            [np.arange(QC * (2 * j + s), QC * (2 * j + s) + QC)
             for j in range(NSLOT)])
        xtq_16 = np.ascontiguousarray(xT16[:, qcols])

        mask_c = np.zeros((NSLOT, P, 1024), np.float32)
        for j in range(NSLOT):
            q0 = QC * (2 * j + s)
            for i in range(4):
                k0 = P * (4 * j + i)
                mask_c[j, :, QC * i:QC * (i + 1)] = (
                    (k0 + kk[:, None]) <= (q0 + qq[None, :]))

        in_maps.append({
            "xt16": xT16, "xtq16": xtq_16, "wall": wall_c,
            "wpj16": wpj_c, "bq": bq_c, "mask": mask_c.astype(np.float16),
        })
    return in_maps


def unshard(results, b_qkv, W_proj, b_proj):
    out = np.zeros((T, C), np.float64)
    for core in range(8):
        s = core % 2
        r = results[core]["out"].astype(np.float64)
        for j in range(NSLOT):
            g0 = QC * (2 * j + s)
            out[g0:g0 + QC] += r[QC * j:QC * (j + 1)]
    # V-bias folded here: (y + b_v) @ W_proj = y @ W_proj + b_v @ W_proj
    b_eff = b_proj.astype(np.float64) + (
        b_qkv[2 * C:3 * C].astype(np.float64) @ W_proj.astype(np.float64))
    out += b_eff
    return out.astype(np.float32).reshape(1, T, C)


_last_result = {}


def kernel(x, mask, W_qkv, b_qkv, W_proj, b_proj):
    from concourse.bass_utils import run_bass_kernel_spmd
    x = np.asarray(x, np.float32)
    W_qkv = np.asarray(W_qkv, np.float32)
    b_qkv = np.asarray(b_qkv, np.float32)
    W_proj = np.asarray(W_proj, np.float32)
    b_proj = np.asarray(b_proj, np.float32)

    if "nc" not in _nc_cache:
        _nc_cache["nc"] = build_nc()
    nc = _nc_cache["nc"]
    in_maps = make_in_maps(x, W_qkv, b_qkv, W_proj)
    import os
    kwargs = {}
    if os.environ.get("BASS_KERNEL_TRACE"):
        kwargs = dict(trace=True, trace_cores=list(range(8)))
    res = run_bass_kernel_spmd(nc, in_maps, core_ids=list(range(8)), **kwargs)
    _last_result["res"] = res
    return unshard([r for r in res.results], b_qkv, W_proj, b_proj)
